# revision 9
# baseline (speedup 1.0000x reference)
"""Expert-parallel grouped GEMM (MoE) kernel for Trainium2.

Problem: out[e] = gelu(tok[e] @ w1[e]) @ w2[e]  per expert e.
  tok: [128, 2048, 128] f32, w1: [128, 128, 512] f32, w2: [128, 512, 128] f32.

Sharding: expert-parallel across 8 NeuronCores, 16 experts per core, no
cross-core communication. Each core runs the same Bass program on its own
expert slice (SPMD), the host concatenates the per-core outputs.

Per-core dataflow (v3):
  - tokens SWDGE-cast f32->bf16 on load, [128 p, 16 m, 128 d] (token t = p*16+m)
  - token transpose to [d, t]: one batched DMA-transpose (X-bar) per expert
    (cfg tok_path="dmat"), or PE transposes + DVE copies (cfg "pe")
  - GEMM1 on PE: w1 bf16 stationary (FWL), tokT moving, N=512 full rate
  - GELU on ACT in groups of `gelu_group` psum banks per instruction
    (amortizes the per-instruction fixed overhead), writes one big bf16
    SBUF tile hsb [128, 4*2048] per expert
  - GEMM2 "direct": stationary = hT 128-token block, moving = w2 tile, psum
    accumulates [t, o] directly -> no output transposes, single DVE drain copy
  - batched per-expert store [128 p, 16 m, 128 o] (HWDGE)
"""

import numpy as np

NUM_CORES = 8
E_TOTAL = 128
E_PER_CORE = E_TOTAL // NUM_CORES  # 16
T = 2048
D = 128
H = 512
O = 128
P = 128

N_BLKS = T // P  # 16 token blocks per expert
N_CHUNKS = 4
BLKS_PER_CHUNK = N_BLKS // N_CHUNKS  # 4
T_CHUNK = T // N_CHUNKS  # 512
H_TILES = H // P  # 4

_CACHE = {}


DEFAULT_CFG = dict(
    # "dmat": batched DMA-transpose; "pe": PE transposes; "host": tokens
    # arrive pre-transposed [E, D, T] with column order t = (m, p), m=t%16
    tok_path="pe",
    g2="classic",  # "direct": hT stationary, [t,o] psum; "classic": w2 stationary
    gelu_group=3,  # psum banks per ACT gelu instruction
    load_ahead=2,
    ph_bufs=2,
    po_bufs=2,
    pt_bufs=2,
    pot_bufs=2,
    osb_bufs=2,
    tokn_bufs=3,
    tokt_bufs=3,
    h_bufs=2,
    outsb_bufs=3,
    w_bufs=3,
)


def _build(loop=1, cfg=None):
    import concourse.bacc as bacc
    import concourse.mybir as mybir
    import concourse.tile as tile
    from concourse.masks import make_identity

    f32 = mybir.dt.float32
    bf16 = mybir.dt.bfloat16
    GELU = mybir.ActivationFunctionType.Gelu
    C = dict(DEFAULT_CFG)
    if cfg:
        C.update(cfg)

    E = E_PER_CORE
    GG = C["gelu_group"]
    N_TILES = H_TILES * N_CHUNKS  # 16 (hd, c) psum tiles per expert

    from contextlib import ExitStack

    nc = bacc.Bacc(
        "TRN2",
        target_bir_lowering=False,
        debug=False,
        num_devices=NUM_CORES,
    )

    host_t = C["tok_path"] == "host"
    tok_shape = [E, D, T] if host_t else [E, T, D]
    tok = nc.dram_tensor("group_token", tok_shape, f32, kind="ExternalInput").ap()
    w1 = nc.dram_tensor("weights1", [E, D, H], f32, kind="ExternalInput").ap()
    w2 = nc.dram_tensor("weights2", [E, H, O], f32, kind="ExternalInput").ap()
    out = nc.dram_tensor("out", [E, T, O], f32, kind="ExternalOutput").ap()

    with tile.TileContext(nc) as tc:
        with ExitStack() as stack:
            const_pool = stack.enter_context(tc.tile_pool(name="const", bufs=1))
            w_pool = stack.enter_context(tc.tile_pool(name="weights", bufs=C["w_bufs"]))
            tokn_pool = stack.enter_context(tc.tile_pool(name="tokn", bufs=C["tokn_bufs"]))
            tokt_pool = stack.enter_context(tc.tile_pool(name="tokt", bufs=C["tokt_bufs"]))
            h_pool = stack.enter_context(tc.tile_pool(name="hts", bufs=C["h_bufs"]))
            outsb_pool = stack.enter_context(tc.tile_pool(name="outsb", bufs=C["outsb_bufs"]))
            ph_pool = stack.enter_context(tc.tile_pool(name="ph", bufs=C["ph_bufs"], space="PSUM"))
            po_pool = stack.enter_context(tc.tile_pool(name="po", bufs=C["po_bufs"], space="PSUM"))

            need_ident = C["tok_path"] == "pe" or C["g2"] == "classic"
            if need_ident:
                ident_f32 = const_pool.tile([P, P], f32)
                make_identity(nc, ident_f32)
                ident = const_pool.tile([P, P], bf16)
                nc.vector.tensor_copy(ident[:], ident_f32[:])

            if C["tok_path"] == "pe":
                pt_pool = stack.enter_context(
                    tc.tile_pool(name="pt", bufs=C["pt_bufs"], space="PSUM")
                )
            if C["g2"] == "classic":
                pot_pool = stack.enter_context(
                    tc.tile_pool(name="pot", bufs=C["pot_bufs"], space="PSUM")
                )
                osb_pool = stack.enter_context(
                    tc.tile_pool(name="osb", bufs=C["osb_bufs"])
                )

            def body(_iv=None):
                tokn = {}
                tokT = {}
                hsb = {}

                def load(e):
                    if host_t:
                        # tokens already [D, T] with col order (m, p); cast-load
                        # straight into the transposed SBUF layout
                        tt = tokt_pool.tile(
                            [P, N_BLKS, P], bf16, tag="tokt", name=f"tokt{e}"
                        )
                        nc.gpsimd.dma_start(
                            tt[:], tok[e].rearrange("d (m p) -> d m p", p=P)
                        )
                        tokT[e] = tt
                    else:
                        tkn = tokn_pool.tile(
                            [P, N_BLKS, D], bf16, tag="tokn", name=f"tokn{e}"
                        )
                        nc.gpsimd.dma_start(
                            tkn[:], tok[e].rearrange("(p m) d -> p m d", p=P)
                        )
                        tokn[e] = tkn
                    w1bf = w_pool.tile([P, H], bf16, tag="w1", name=f"w1b{e}")
                    nc.gpsimd.dma_start(w1bf[:], w1[e])
                    w2bf = w_pool.tile([P, H_TILES, O], bf16, tag="w2", name=f"w2b{e}")
                    nc.gpsimd.dma_start(
                        w2bf[:], w2[e].rearrange("(k p) o -> p k o", p=P)
                    )
                    tokn[e, "w"] = (w1bf, w2bf)

                def tin(e):
                    if host_t:
                        return
                    # tokT[d, m, p]: token t = p*16 + m lives at column m*128+p
                    tt = tokt_pool.tile([P, N_BLKS, P], bf16, tag="tokt", name=f"tokt{e}")
                    if C["tok_path"] == "dmat":
                        nc.sync.dma_start(
                            tt[:],
                            tokn[e][:].rearrange("p m d -> p (m d)"),
                            transpose=True,
                        )
                    else:
                        for c in range(N_CHUNKS):
                            pt = pt_pool.tile([P, T_CHUNK], bf16, tag="pt")
                            for j in range(BLKS_PER_CHUNK):
                                nc.tensor.transpose(
                                    pt[:, j * P : (j + 1) * P],
                                    tokn[e][:, c * BLKS_PER_CHUNK + j],
                                    ident[:],
                                )
                            nc.vector.tensor_copy(
                                tt[:, c * BLKS_PER_CHUNK : (c + 1) * BLKS_PER_CHUNK],
                                pt[:].rearrange("p (m q) -> p m q", m=BLKS_PER_CHUNK),
                            )
                    tokT[e] = tt

                def g1(e):
                    w1bf, _ = tokn[e, "w"]
                    # hsb columns: flat = (hd*4 + c)*512 + i
                    hs = h_pool.tile([P, H_TILES * T], bf16, tag="hsb", name=f"hsb{e}")
                    hsb[e] = hs
                    tt = tokT[e][:].rearrange("p m q -> p (m q)")
                    ph = None
                    base = 0
                    for flat in range(N_TILES):
                        hd, c = divmod(flat, N_CHUNKS)
                        i = flat % GG
                        if i == 0:
                            gsz = min(GG, N_TILES - flat)
                            ph = ph_pool.tile([P, gsz, T_CHUNK], f32, tag="ph")
                            base = flat
                        nc.tensor.matmul(
                            ph[:, i],
                            w1bf[:, hd * P : (hd + 1) * P],
                            tt[:, c * T_CHUNK : (c + 1) * T_CHUNK],
                            start=True,
                            stop=True,
                        )
                        if i == gsz - 1:
                            nc.scalar.activation(
                                hs[:, base * T_CHUNK : (flat + 1) * T_CHUNK],
                                ph[:].rearrange("p g q -> p (g q)"),
                                GELU,
                            )

                def g2_direct(e):
                    _, w2bf = tokn[e, "w"]
                    hs = hsb[e]
                    osb = outsb_pool.tile([P, N_BLKS, O], f32, tag="outsb", name=f"osb{e}")
                    for c in range(N_CHUNKS):
                        po = po_pool.tile([P, BLKS_PER_CHUNK, O], f32, tag="po")
                        for j in range(BLKS_PER_CHUNK):
                            m = c * BLKS_PER_CHUNK + j
                            for hd in range(H_TILES):
                                nc.tensor.matmul(
                                    po[:, j],
                                    hs[:, (hd * N_BLKS + m) * P : (hd * N_BLKS + m + 1) * P],
                                    w2bf[:, hd],
                                    start=(hd == 0),
                                    stop=(hd == H_TILES - 1),
                                )
                        nc.vector.tensor_copy(
                            osb[:, c * BLKS_PER_CHUNK : (c + 1) * BLKS_PER_CHUNK],
                            po[:],
                        )
                    return osb

                def g2_classic(e):
                    _, w2bf = tokn[e, "w"]
                    hs = hsb[e]
                    osb_out = outsb_pool.tile(
                        [P, N_BLKS, O], f32, tag="outsb", name=f"osb{e}"
                    )
                    for c in range(N_CHUNKS):
                        po = po_pool.tile([P, T_CHUNK], f32, tag="po")
                        for hd in range(H_TILES):
                            nc.tensor.matmul(
                                po[:],
                                w2bf[:, hd],
                                hs[:, (hd * N_CHUNKS + c) * T_CHUNK : (hd * N_CHUNKS + c + 1) * T_CHUNK],
                                start=(hd == 0),
                                stop=(hd == H_TILES - 1),
                            )
                        ob = osb_pool.tile([P, T_CHUNK], bf16, tag="ob")
                        nc.vector.tensor_copy(ob[:], po[:])
                        pot = pot_pool.tile([P, T_CHUNK], bf16, tag="pot")
                        for j in range(BLKS_PER_CHUNK):
                            nc.tensor.transpose(
                                pot[:, j * P : (j + 1) * P],
                                ob[:, j * P : (j + 1) * P],
                                ident[:],
                            )
                        nc.vector.tensor_copy(
                            osb_out[:, c * BLKS_PER_CHUNK : (c + 1) * BLKS_PER_CHUNK],
                            pot[:].rearrange("p (m q) -> p m q", m=BLKS_PER_CHUNK),
                        )
                    return osb_out

                g2 = g2_direct if C["g2"] == "direct" else g2_classic

                def store(e, osb):
                    nc.sync.dma_start(
                        out[e].rearrange("(p m) o -> p m o", p=P), osb[:]
                    )

                LA = C["load_ahead"]
                for e in range(min(LA, E)):
                    load(e)
                tin(0)
                pending = {}
                for e in range(E):
                    if e + LA < E:
                        load(e + LA)
                    if e + 1 < E:
                        tin(e + 1)
                    g1(e)
                    if e > 0:
                        pending[e - 1] = g2(e - 1)
                        store(e - 1, pending.pop(e - 1))
                pending[E - 1] = g2(E - 1)
                store(E - 1, pending.pop(E - 1))

            if loop == 1:
                body()
            else:
                with tc.For_i(0, loop, 1) as _i:
                    body(_i)

    nc.compile()
    return nc


def _get_nc(loop=1, cfg=None):
    key = ("nc", loop, tuple(sorted((cfg or {}).items())))
    if key not in _CACHE:
        _CACHE[key] = _build(loop, cfg)
    return _CACHE[key]


ACTIVE_CFG = None  # overrides DEFAULT_CFG for kernel() when set


def host_transpose_tokens(tok_slice):
    """[E, T, D] -> [E, D, T] with column order t = (m, p), token t = p*16+m."""
    E = tok_slice.shape[0]
    return np.ascontiguousarray(
        tok_slice.reshape(E, P, N_BLKS, D).transpose(0, 3, 2, 1).reshape(E, D, T)
    )


def make_in_maps(group_token, weights1, weights2, cfg=None):
    C = dict(DEFAULT_CFG)
    if cfg:
        C.update(cfg)
    host_t = C["tok_path"] == "host"
    in_maps = []
    for c in range(NUM_CORES):
        sl = slice(c * E_PER_CORE, (c + 1) * E_PER_CORE)
        tok_c = group_token[sl]
        tok_c = (
            host_transpose_tokens(tok_c)
            if host_t
            else np.ascontiguousarray(tok_c)
        )
        in_maps.append(
            {
                "group_token": tok_c,
                "weights1": np.ascontiguousarray(weights1[sl]),
                "weights2": np.ascontiguousarray(weights2[sl]),
            }
        )
    return in_maps


def kernel(group_token, weights1, weights2):
    from concourse.bass_utils import run_bass_kernel_spmd

    group_token = np.asarray(group_token, dtype=np.float32)
    weights1 = np.asarray(weights1, dtype=np.float32)
    weights2 = np.asarray(weights2, dtype=np.float32)

    nc = _get_nc(cfg=ACTIVE_CFG)
    in_maps = make_in_maps(group_token, weights1, weights2, ACTIVE_CFG)

    res = run_bass_kernel_spmd(nc, in_maps, core_ids=list(range(NUM_CORES)))
    _CACHE["last_results"] = res
    return np.concatenate([r["out"] for r in res.results], axis=0)
